# revision 7
# baseline (speedup 1.0000x reference)
"""BondInfluenceSelfAttention TRN2 kernel.

Full-input contract: kernel(**inputs) takes the complete unsharded inputs and
returns the full [B, L, D] output. Internally shards across 8 NeuronCores:
core c handles batch b = c // 4 and head-group g = c % 4 (4 heads, 256 dk dims).
Each core computes its heads' attention plus the partial output projection
through its 256 rows of Wo; the host sums the 4 partials per batch and adds bo.

Device-side formulation (per core), all matmuls in fp32r:
  QT = (Wq_g/8)^T x^T   [256, L]   (1/sqrt(dk)=1/8 folded into Wq/bq on host)
  KT = Wk_g^T x^T       [256, L]
  V  = x Wv_g           [L, 256]   (bias via an appended ones-row matmul)
  S^T tile = K Q^T      [L_k, L_q] (per head, dk=64 contraction)
  P~ = exp(S^T * bondT) (no max subtraction: |scores| <~ 3, fp32 exp is safe)
  O^T accumulated per head pair with column packing (even head -> psum rows
  0:64, odd head -> rows 64:128); softmax denominators accumulated with M=1
  ones matmuls col-tiled to psum rows {0,32,64,96} of one bank. Reciprocals
  stay on their own lanes; K=1 broadcast matmuls expand them to [128, 512]
  for the normalize multiply. Finally Y = O Wo_g.
"""

import numpy as np

try:
    import concourse.bass as bass  # noqa: F401
except ImportError:  # pragma: no cover
    import sys

    sys.path.insert(0, "/opt/trn_rl_repo")

import concourse.bacc as bacc
import concourse.mybir as mybir
import concourse.tile as tile
from concourse.bass_utils import run_bass_kernel_spmd

F32 = mybir.dt.float32
F32R = mybir.dt.float32r

D = 1024  # d_model
L = 2048  # sequence length
B = 2  # batch
HPC = 4  # heads per core
DKG = 256  # dk dims per core (4 heads x 64)
NK = D // 128  # 8 contraction k-tiles for the projections
LT = L // 128  # 16 L-tiles
NCH = L // 512  # 4 L_q chunks
N_CORES = 8

_CACHED_NC = None


def _build_nc():
    nc = bacc.Bacc("TRN2", target_bir_lowering=False, debug=False, num_devices=N_CORES)

    xt_d = nc.declare_dram_parameter("xt", [D, L], F32R, isOutput=False)
    bd_d = nc.declare_dram_parameter("bd", [L, L], F32, isOutput=False)
    wq_d = nc.declare_dram_parameter("wq", [D, DKG], F32R, isOutput=False)
    wk_d = nc.declare_dram_parameter("wk", [D, DKG], F32R, isOutput=False)
    wv_d = nc.declare_dram_parameter("wv", [D, DKG], F32R, isOutput=False)
    bqk_d = nc.declare_dram_parameter("bqk", [128, 4], F32, isOutput=False)
    bv_d = nc.declare_dram_parameter("bv", [1, DKG], F32R, isOutput=False)
    wo_d = nc.declare_dram_parameter("wo", [DKG, D], F32R, isOutput=False)
    y_d = nc.declare_dram_parameter("y", [L, D], F32, isOutput=True)

    Exp = mybir.ActivationFunctionType.Exp

    with tile.TileContext(nc) as tc:
        with tc.tile_pool(name="persist", bufs=1) as pp:
            qt = [pp.tile([128, L], F32R, tag=f"qt{t}", name=f"qt{t}") for t in range(2)]
            kt = [pp.tile([128, L], F32R, tag=f"kt{t}", name=f"kt{t}") for t in range(2)]
            vt = [
                pp.tile([128, HPC, 65], F32R, tag=f"v{i}", name=f"v{i}")
                for i in range(LT)
            ]
            ot = [pp.tile([128, L], F32R, tag=f"ot{t}", name=f"ot{t}") for t in range(2)]
            wo_sb = pp.tile([128, 2, D], F32R, tag="wo", name="wo_sb")
            bqk_sb = pp.tile([128, 4], F32, tag="bqk", name="bqk_sb")
            bv_sb = pp.tile([1, DKG], F32R, tag="bv", name="bv_sb")
            onesv_f = pp.tile([1, 128], F32, tag="onesvf", name="onesv_f")
            onesv = pp.tile([1, 128], F32R, tag="onesv", name="onesv")
            onesb_f = pp.tile([128, 128], F32, tag="onesbf", name="onesb_f")
            onesb = pp.tile([128, 128], F32R, tag="onesb", name="onesb")

            nc.sync.dma_start(out=wo_sb, in_=wo_d.ap().rearrange("(t p) n -> p t n", p=128))
            nc.sync.dma_start(out=bqk_sb, in_=bqk_d[:, :])
            nc.sync.dma_start(out=bv_sb, in_=bv_d[:, :])
            nc.vector.memset(onesv_f, 1.0)
            nc.vector.tensor_copy(out=onesv, in_=onesv_f)
            nc.vector.memset(onesb_f, 1.0)
            nc.vector.tensor_copy(out=onesb, in_=onesb_f)

            # ---------------- Phase 1: projections ----------------
            with tc.tile_pool(name="xw", bufs=1) as xw, tc.tile_pool(
                name="ps1", bufs=2, space="PSUM"
            ) as ps1:
                xk = [
                    xw.tile([128, L], F32R, tag=f"x{k}", name=f"x{k}") for k in range(NK)
                ]
                xt_t = xt_d.ap().rearrange("(k p) l -> k p l", p=128)
                for k in range(NK):
                    nc.sync.dma_start(out=xk[k], in_=xt_t[k])
                wq_sb = xw.tile([128, NK, DKG], F32R, tag="wq", name="wq_sb")
                wk_sb = xw.tile([128, NK, DKG], F32R, tag="wk", name="wk_sb")
                wv_sb = xw.tile([128, NK, DKG], F32R, tag="wv", name="wv_sb")
                nc.sync.dma_start(out=wq_sb, in_=wq_d.ap().rearrange("(k p) n -> p k n", p=128))
                nc.sync.dma_start(out=wk_sb, in_=wk_d.ap().rearrange("(k p) n -> p k n", p=128))
                nc.sync.dma_start(out=wv_sb, in_=wv_d.ap().rearrange("(k p) n -> p k n", p=128))

                for t in range(2):
                    for c in range(NCH):
                        pq = ps1.tile([128, 512], F32, tag="pq", name="pq")
                        for k in range(NK):
                            nc.tensor.matmul(
                                pq[:, :],
                                wq_sb[:, k, 128 * t : 128 * (t + 1)],
                                xk[k][:, 512 * c : 512 * (c + 1)],
                                start=(k == 0),
                                stop=(k == NK - 1),
                            )
                        nc.vector.tensor_scalar_add(
                            out=qt[t][:, 512 * c : 512 * (c + 1)],
                            in0=pq[:, :],
                            scalar1=bqk_sb[:, t : t + 1],
                        )
                        pk = ps1.tile([128, 512], F32, tag="pk", name="pk")
                        for k in range(NK):
                            nc.tensor.matmul(
                                pk[:, :],
                                wk_sb[:, k, 128 * t : 128 * (t + 1)],
                                xk[k][:, 512 * c : 512 * (c + 1)],
                                start=(k == 0),
                                stop=(k == NK - 1),
                            )
                        nc.vector.tensor_scalar_add(
                            out=kt[t][:, 512 * c : 512 * (c + 1)],
                            in0=pk[:, :],
                            scalar1=bqk_sb[:, 2 + t : 3 + t],
                        )

                for i in range(LT):
                    pv = ps1.tile([128, DKG], F32, tag="pv", name="pv")
                    for k in range(NK):
                        nc.tensor.matmul(
                            pv[:, :],
                            xk[k][:, 128 * i : 128 * (i + 1)],
                            wv_sb[:, k, :],
                            start=(k == 0),
                            stop=False,
                        )
                    nc.tensor.matmul(
                        pv[:, :], onesv[:, :], bv_sb[:, :], start=False, stop=True
                    )
                    nc.vector.tensor_copy(
                        out=vt[i][:, :, 0:64],
                        in_=pv.rearrange("p (h e) -> p h e", e=64),
                    )
                    nc.vector.memset(vt[i][:, :, 64:65].bitcast(F32), 1.0)
                    nc.vector.tensor_copy(
                        out=vt[i][:, :, 64:65], in_=vt[i][:, :, 64:65].bitcast(F32)
                    )

            # ---------------- Phase 2: attention ----------------
            with tc.tile_pool(name="att", bufs=1) as att, tc.tile_pool(
                name="ps2", bufs=1, space="PSUM"
            ) as ps2:
                for c in range(NCH):
                    oaccs = [
                        ps2.tile([65, 512], F32, tag="oacc", bufs=4, name=f"oacc{h}")
                        for h in range(HPC)
                    ]
                    for i in range(LT):
                        bt = att.tile([128, 512], F32, tag="bond", bufs=3, name="bt")
                        nc.sync.dma_start(
                            out=bt,
                            in_=bd_d[128 * i : 128 * (i + 1), 512 * c : 512 * (c + 1)],
                        )
                        for h in range(HPC):
                            t, half = h // 2, h % 2
                            sp = ps2.tile([128, 512], F32, tag="s", bufs=3, name="sp")
                            nc.tensor.matmul(
                                sp[:, :],
                                kt[t][64 * half : 64 * (half + 1), 128 * i : 128 * (i + 1)],
                                qt[t][64 * half : 64 * (half + 1), 512 * c : 512 * (c + 1)],
                                start=True,
                                stop=True,
                            )
                            nc.vector.tensor_mul(out=sp[:, :], in0=sp[:, :], in1=bt)
                            ptile = att.tile([128, 512], F32R, tag="pt", bufs=4, name="ptile")
                            nc.scalar.activation(out=ptile, in_=sp[:, :], func=Exp)
                            nc.tensor.matmul(
                                oaccs[h][:, :],
                                vt[i][:, h, :],
                                ptile,
                                start=(i == 0),
                                stop=(i == LT - 1),
                            )
                    # chunk tail: denominators sit on lane 64 of each oacc
                    rd = att.tile([65, HPC, 512], F32R, tag="rd", bufs=2, name="rd")
                    with nc.allow_low_precision(reason="f32r is full fp32 storage"):
                        for h in range(HPC):
                            nc.vector.reciprocal(
                                out=rd[64:65, h, :],
                                in_=oaccs[h][64:65, :],
                            )
                    for t in range(2):
                        for half in range(2):
                            h = 2 * t + half
                            bc = ps2.tile([64, 512], F32, tag="s", bufs=3, name="bc")
                            nc.tensor.matmul(
                                bc[:, :],
                                onesb[64:65, 0:64],
                                rd[64:65, h, :],
                                start=True,
                                stop=True,
                                tile_position=(64, 0),
                            )
                            bcs = att.tile([64, 512], F32, tag="bcs", bufs=3, name="bcs")
                            nc.vector.tensor_copy(out=bcs, in_=bc[:, :])
                            if half == 0:
                                nc.vector.tensor_mul(
                                    out=ot[t][0:64, 512 * c : 512 * (c + 1)],
                                    in0=oaccs[h][0:64, :],
                                    in1=bcs,
                                )
                            else:
                                odd = att.tile([64, 512], F32R, tag="odd", bufs=2, name="odd")
                                nc.vector.tensor_mul(
                                    out=odd,
                                    in0=oaccs[h][0:64, :],
                                    in1=bcs,
                                )
                                nc.sync.dma_start(
                                    out=ot[t][64:128, 512 * c : 512 * (c + 1)],
                                    in_=odd,
                                )

            # ---------------- Phase 3: output projection ----------------
            with tc.tile_pool(name="fin", bufs=1) as fin, tc.tile_pool(
                name="ps3", bufs=4, space="PSUM"
            ) as ps3:
                for j in range(LT):
                    for dh in range(2):
                        yp = ps3.tile([128, 512], F32, tag="y", name="yp")
                        for t in range(2):
                            nc.tensor.matmul(
                                yp[:, :],
                                ot[t][:, 128 * j : 128 * (j + 1)],
                                wo_sb[:, t, 512 * dh : 512 * (dh + 1)],
                                start=(t == 0),
                                stop=(t == 1),
                            )
                        ys = fin.tile([128, 512], F32, tag="ys", bufs=3, name="ys")
                        nc.vector.tensor_copy(out=ys, in_=yp[:, :])
                        nc.sync.dma_start(
                            out=y_d[128 * j : 128 * (j + 1), 512 * dh : 512 * (dh + 1)],
                            in_=ys,
                        )

    nc.compile()
    return nc


def _get_nc():
    global _CACHED_NC
    if _CACHED_NC is None:
        _CACHED_NC = _build_nc()
    return _CACHED_NC


def _host_prep(x, bond_influence, Wq, bq, Wk, bk, Wv, bv, Wo):
    in_maps = []
    for core in range(N_CORES):
        b, g = core // HPC, core % HPC
        s = slice(g * DKG, (g + 1) * DKG)
        bq_g = (bq[s] / 8.0).astype(np.float32)
        bk_g = bk[s].astype(np.float32)
        bqk = np.stack(
            [bq_g[0:128], bq_g[128:256], bk_g[0:128], bk_g[128:256]], axis=1
        )
        in_maps.append(
            {
                "xt": np.ascontiguousarray(x[b].T),
                "bd": np.ascontiguousarray(bond_influence[b].T),
                "wq": np.ascontiguousarray(Wq[:, s] / 8.0),
                "wk": np.ascontiguousarray(Wk[:, s]),
                "wv": np.ascontiguousarray(Wv[:, s]),
                "bqk": np.ascontiguousarray(bqk),
                "bv": np.ascontiguousarray(bv[s][None, :]),
                "wo": np.ascontiguousarray(Wo[s, :]),
            }
        )
    return in_maps


def kernel(
    x,
    bond_influence,
    Wq,
    bq,
    Wk,
    bk,
    Wv,
    bv,
    Wo,
    bo,
    _trace=False,
    _trace_out=None,
):
    x = np.asarray(x, dtype=np.float32)
    bond_influence = np.asarray(bond_influence, dtype=np.float32)
    args = [np.asarray(a, dtype=np.float32) for a in (Wq, bq, Wk, bk, Wv, bv, Wo)]
    bo = np.asarray(bo, dtype=np.float32)

    nc = _get_nc()
    in_maps = _host_prep(x, bond_influence, *args)
    kwargs = {}
    if _trace:
        kwargs = dict(trace=True, tmpdir=_trace_out)
    res = run_bass_kernel_spmd(nc, in_maps, list(range(N_CORES)), **kwargs)

    out = np.zeros((B, L, D), dtype=np.float32)
    for b in range(B):
        acc = res.results[4 * b]["y"].astype(np.float32).copy()
        for g in range(1, HPC):
            acc += res.results[4 * b + g]["y"]
        out[b] = acc + bo[None, :]
    if _trace:
        return out, res
    return out


# revision 8
# speedup vs baseline: 1.2093x; 1.2093x over previous
"""BondInfluenceSelfAttention TRN2 kernel.

Full-input contract: kernel(**inputs) takes the complete unsharded inputs and
returns the full [B, L, D] output. Internally shards across 8 NeuronCores:
core c handles batch b = c // 4 and head-group g = c % 4 (4 heads, 256 dk dims).
Each core computes its heads' attention plus the partial output projection
through its 256 rows of Wo; the host sums the 4 partials per batch and adds bo.

Device-side formulation (per core), all matmuls in fp32r:
  QT = (Wq_g/8)^T x^T   [256, L]   (1/sqrt(dk)=1/8 folded into Wq/bq on host)
  KT = Wk_g^T x^T       [256, L]
  V  = x Wv_g           [L, 256]   (bias via an appended ones-row matmul)
  S^T tile = K Q^T      [L_k, L_q] (per head, dk=64 contraction)
  P~ = exp(S^T * bondT) (no max subtraction: |scores| <~ 3, fp32 exp is safe)
  O^T accumulated per head pair with column packing (even head -> psum rows
  0:64, odd head -> rows 64:128); softmax denominators accumulated with M=1
  ones matmuls col-tiled to psum rows {0,32,64,96} of one bank. Reciprocals
  stay on their own lanes; K=1 broadcast matmuls expand them to [128, 512]
  for the normalize multiply. Finally Y = O Wo_g.
"""

import numpy as np

try:
    import concourse.bass as bass  # noqa: F401
except ImportError:  # pragma: no cover
    import sys

    sys.path.insert(0, "/opt/trn_rl_repo")

import concourse.bacc as bacc
import concourse.mybir as mybir
import concourse.tile as tile
from concourse.bass_utils import run_bass_kernel_spmd

F32 = mybir.dt.float32
F32R = mybir.dt.float32r

D = 1024  # d_model
L = 2048  # sequence length
B = 2  # batch
HPC = 4  # heads per core
DKG = 256  # dk dims per core (4 heads x 64)
NK = D // 128  # 8 contraction k-tiles for the projections
LT = L // 128  # 16 L-tiles
NCH = L // 512  # 4 L_q chunks
N_CORES = 8

_CACHED_NC = None


def _build_nc():
    nc = bacc.Bacc("TRN2", target_bir_lowering=False, debug=False, num_devices=N_CORES)

    xt_d = nc.declare_dram_parameter("xt", [D, L], F32R, isOutput=False)
    bd_d = nc.declare_dram_parameter("bd", [L, L], F32, isOutput=False)
    wq_d = nc.declare_dram_parameter("wq", [D, DKG], F32R, isOutput=False)
    wk_d = nc.declare_dram_parameter("wk", [D, DKG], F32R, isOutput=False)
    wv_d = nc.declare_dram_parameter("wv", [D, DKG], F32R, isOutput=False)
    bqk_d = nc.declare_dram_parameter("bqk", [128, 4], F32, isOutput=False)
    bv_d = nc.declare_dram_parameter("bv", [1, DKG], F32R, isOutput=False)
    wo_d = nc.declare_dram_parameter("wo", [DKG, D], F32R, isOutput=False)
    y_d = nc.declare_dram_parameter("y", [L, D], F32, isOutput=True)

    Exp = mybir.ActivationFunctionType.Exp
    Identity = mybir.ActivationFunctionType.Identity

    with tile.TileContext(nc) as tc:
        with tc.tile_pool(name="persist", bufs=1) as pp:
            qt = [pp.tile([128, L], F32R, tag=f"qt{t}", name=f"qt{t}") for t in range(2)]
            kt = [pp.tile([128, L], F32R, tag=f"kt{t}", name=f"kt{t}") for t in range(2)]
            vt = [
                pp.tile([128, HPC, 65], F32R, tag=f"v{i}", name=f"v{i}")
                for i in range(LT)
            ]
            ot = [pp.tile([128, L], F32R, tag=f"ot{t}", name=f"ot{t}") for t in range(2)]
            wo_sb = pp.tile([128, 2, D], F32R, tag="wo", name="wo_sb")
            bqk_sb = pp.tile([128, 4], F32, tag="bqk", name="bqk_sb")
            bv_sb = pp.tile([1, DKG], F32R, tag="bv", name="bv_sb")
            onesv_f = pp.tile([1, 128], F32, tag="onesvf", name="onesv_f")
            onesv = pp.tile([1, 128], F32R, tag="onesv", name="onesv")
            onesb_f = pp.tile([128, 128], F32, tag="onesbf", name="onesb_f")
            onesb = pp.tile([128, 128], F32R, tag="onesb", name="onesb")

            nc.sync.dma_start(out=wo_sb, in_=wo_d.ap().rearrange("(t p) n -> p t n", p=128))
            nc.sync.dma_start(out=bqk_sb, in_=bqk_d[:, :])
            nc.sync.dma_start(out=bv_sb, in_=bv_d[:, :])
            nc.vector.memset(onesv_f, 1.0)
            nc.vector.tensor_copy(out=onesv, in_=onesv_f)
            nc.vector.memset(onesb_f, 1.0)
            nc.vector.tensor_copy(out=onesb, in_=onesb_f)

            # ---------------- Phase 1: projections ----------------
            with tc.tile_pool(name="xw", bufs=1) as xw, tc.tile_pool(
                name="ps1", bufs=2, space="PSUM"
            ) as ps1:
                xk = [
                    xw.tile([128, L], F32R, tag=f"x{k}", name=f"x{k}") for k in range(NK)
                ]
                xt_t = xt_d.ap().rearrange("(k p) l -> k p l", p=128)
                for k in range(NK):
                    nc.sync.dma_start(out=xk[k], in_=xt_t[k])
                wq_sb = xw.tile([128, NK, DKG], F32R, tag="wq", name="wq_sb")
                wk_sb = xw.tile([128, NK, DKG], F32R, tag="wk", name="wk_sb")
                wv_sb = xw.tile([128, NK, DKG], F32R, tag="wv", name="wv_sb")
                nc.sync.dma_start(out=wq_sb, in_=wq_d.ap().rearrange("(k p) n -> p k n", p=128))
                nc.sync.dma_start(out=wk_sb, in_=wk_d.ap().rearrange("(k p) n -> p k n", p=128))
                nc.sync.dma_start(out=wv_sb, in_=wv_d.ap().rearrange("(k p) n -> p k n", p=128))

                for t in range(2):
                    for c in range(NCH):
                        pq = ps1.tile([128, 512], F32, tag="pq", name="pq")
                        for k in range(NK):
                            nc.tensor.matmul(
                                pq[:, :],
                                wq_sb[:, k, 128 * t : 128 * (t + 1)],
                                xk[k][:, 512 * c : 512 * (c + 1)],
                                start=(k == 0),
                                stop=(k == NK - 1),
                            )
                        nc.scalar.activation(
                            out=qt[t][:, 512 * c : 512 * (c + 1)],
                            in_=pq[:, :],
                            func=Identity,
                            bias=bqk_sb[:, t : t + 1],
                        )
                        pk = ps1.tile([128, 512], F32, tag="pk", name="pk")
                        for k in range(NK):
                            nc.tensor.matmul(
                                pk[:, :],
                                wk_sb[:, k, 128 * t : 128 * (t + 1)],
                                xk[k][:, 512 * c : 512 * (c + 1)],
                                start=(k == 0),
                                stop=(k == NK - 1),
                            )
                        nc.scalar.activation(
                            out=kt[t][:, 512 * c : 512 * (c + 1)],
                            in_=pk[:, :],
                            func=Identity,
                            bias=bqk_sb[:, 2 + t : 3 + t],
                        )

                for i in range(LT):
                    pv = ps1.tile([128, DKG], F32, tag="pv", name="pv")
                    for k in range(NK):
                        nc.tensor.matmul(
                            pv[:, :],
                            xk[k][:, 128 * i : 128 * (i + 1)],
                            wv_sb[:, k, :],
                            start=(k == 0),
                            stop=False,
                        )
                    nc.tensor.matmul(
                        pv[:, :], onesv[:, :], bv_sb[:, :], start=False, stop=True
                    )
                    nc.vector.tensor_copy(
                        out=vt[i][:, :, 0:64],
                        in_=pv.rearrange("p (h e) -> p h e", e=64),
                    )
                    nc.vector.memset(vt[i][:, :, 64:65].bitcast(F32), 1.0)
                    nc.vector.tensor_copy(
                        out=vt[i][:, :, 64:65], in_=vt[i][:, :, 64:65].bitcast(F32)
                    )

            # ------- Phase 2+3: attention with interleaved output projection -------
            with tc.tile_pool(name="att", bufs=1) as att, tc.tile_pool(
                name="ps2", bufs=1, space="PSUM"
            ) as ps2:
                for c in range(NCH):
                    oaccs = [
                        ps2.tile([65, 512], F32, tag="oacc", bufs=4, name=f"oacc{h}")
                        for h in range(HPC)
                    ]
                    for i in range(LT):
                        bt = att.tile([128, 512], F32, tag="bond", bufs=3, name="bt")
                        nc.sync.dma_start(
                            out=bt,
                            in_=bd_d[128 * i : 128 * (i + 1), 512 * c : 512 * (c + 1)],
                        )
                        for h in range(HPC):
                            t, half = h // 2, h % 2
                            sp = ps2.tile([128, 512], F32, tag="s", bufs=3, name="sp")
                            nc.tensor.matmul(
                                sp[:, :],
                                kt[t][64 * half : 64 * (half + 1), 128 * i : 128 * (i + 1)],
                                qt[t][64 * half : 64 * (half + 1), 512 * c : 512 * (c + 1)],
                                start=True,
                                stop=True,
                            )
                            sb = att.tile([128, 512], F32, tag="sb", bufs=4, name="sb")
                            nc.vector.tensor_mul(out=sb, in0=sp[:, :], in1=bt)
                            ptile = att.tile([128, 512], F32R, tag="pt", bufs=4, name="ptile")
                            nc.scalar.activation(out=ptile, in_=sb, func=Exp)
                            nc.tensor.matmul(
                                oaccs[h][:, :],
                                vt[i][:, h, :],
                                ptile,
                                start=(i == 0),
                                stop=(i == LT - 1),
                            )
                    # chunk tail: denominators sit on lane 64 of each oacc
                    rd = att.tile([65, HPC, 512], F32R, tag="rd", bufs=2, name="rd")
                    with nc.allow_low_precision(reason="f32r is full fp32 storage"):
                        for h in range(HPC):
                            nc.vector.reciprocal(
                                out=rd[64:65, h, :],
                                in_=oaccs[h][64:65, :],
                            )
                    for t in range(2):
                        for half in range(2):
                            h = 2 * t + half
                            bc = ps2.tile([64, 512], F32, tag="s", bufs=3, name="bc")
                            nc.tensor.matmul(
                                bc[:, :],
                                onesb[64:65, 0:64],
                                rd[64:65, h, :],
                                start=True,
                                stop=True,
                                tile_position=(64, 0),
                            )
                            bcs = att.tile([64, 512], F32, tag="bcs", bufs=3, name="bcs")
                            nc.vector.tensor_copy(out=bcs, in_=bc[:, :])
                            if half == 0:
                                nc.vector.tensor_mul(
                                    out=ot[t][0:64, 512 * c : 512 * (c + 1)],
                                    in0=oaccs[h][0:64, :],
                                    in1=bcs,
                                )
                            else:
                                odd = att.tile([64, 512], F32R, tag="odd", bufs=2, name="odd")
                                nc.vector.tensor_mul(
                                    out=odd,
                                    in0=oaccs[h][0:64, :],
                                    in1=bcs,
                                )
                                nc.sync.dma_start(
                                    out=ot[t][64:128, 512 * c : 512 * (c + 1)],
                                    in_=odd,
                                )
                    # output projection for this chunk's four L-tiles
                    for j in range(4 * c, 4 * c + 4):
                        for dh in range(2):
                            yp = ps2.tile([128, 512], F32, tag="y", bufs=1, name="yp")
                            for t in range(2):
                                nc.tensor.matmul(
                                    yp[:, :],
                                    ot[t][:, 128 * j : 128 * (j + 1)],
                                    wo_sb[:, t, 512 * dh : 512 * (dh + 1)],
                                    start=(t == 0),
                                    stop=(t == 1),
                                )
                            ys = att.tile([128, 512], F32, tag="ys", bufs=3, name="ys")
                            if (j + dh) % 2 == 0:
                                nc.vector.tensor_copy(out=ys, in_=yp[:, :])
                            else:
                                nc.scalar.activation(out=ys, in_=yp[:, :], func=Identity)
                            nc.sync.dma_start(
                                out=y_d[128 * j : 128 * (j + 1), 512 * dh : 512 * (dh + 1)],
                                in_=ys,
                            )

    nc.compile()
    return nc


def _get_nc():
    global _CACHED_NC
    if _CACHED_NC is None:
        _CACHED_NC = _build_nc()
    return _CACHED_NC


def _host_prep(x, bond_influence, Wq, bq, Wk, bk, Wv, bv, Wo):
    in_maps = []
    for core in range(N_CORES):
        b, g = core // HPC, core % HPC
        s = slice(g * DKG, (g + 1) * DKG)
        bq_g = (bq[s] / 8.0).astype(np.float32)
        bk_g = bk[s].astype(np.float32)
        bqk = np.stack(
            [bq_g[0:128], bq_g[128:256], bk_g[0:128], bk_g[128:256]], axis=1
        )
        in_maps.append(
            {
                "xt": np.ascontiguousarray(x[b].T),
                "bd": np.ascontiguousarray(bond_influence[b].T),
                "wq": np.ascontiguousarray(Wq[:, s] / 8.0),
                "wk": np.ascontiguousarray(Wk[:, s]),
                "wv": np.ascontiguousarray(Wv[:, s]),
                "bqk": np.ascontiguousarray(bqk),
                "bv": np.ascontiguousarray(bv[s][None, :]),
                "wo": np.ascontiguousarray(Wo[s, :]),
            }
        )
    return in_maps


def kernel(
    x,
    bond_influence,
    Wq,
    bq,
    Wk,
    bk,
    Wv,
    bv,
    Wo,
    bo,
    _trace=False,
    _trace_out=None,
):
    x = np.asarray(x, dtype=np.float32)
    bond_influence = np.asarray(bond_influence, dtype=np.float32)
    args = [np.asarray(a, dtype=np.float32) for a in (Wq, bq, Wk, bk, Wv, bv, Wo)]
    bo = np.asarray(bo, dtype=np.float32)

    nc = _get_nc()
    in_maps = _host_prep(x, bond_influence, *args)
    kwargs = {}
    if _trace:
        kwargs = dict(trace=True, tmpdir=_trace_out)
    res = run_bass_kernel_spmd(nc, in_maps, list(range(N_CORES)), **kwargs)

    out = np.zeros((B, L, D), dtype=np.float32)
    for b in range(B):
        acc = res.results[4 * b]["y"].astype(np.float32).copy()
        for g in range(1, HPC):
            acc += res.results[4 * b + g]["y"]
        out[b] = acc + bo[None, :]
    if _trace:
        return out, res
    return out
